# revision 11
# baseline (speedup 1.0000x reference)
"""Trainium2 Bass kernel for nn_BinaryLabelSoftRouter.

Reference computation (B=16, T=1024, D=2048, H=256, H2=128):
  base   = where(labels>0, [.25,.75], [.75,.25])            # (B,T,2)
  h1     = gelu(LN(x @ W1 + b1) * g1 + be1)                 # erf gelu
  h2     = gelu(LN(h1 @ W2 + b2) * g2 + be2)
  adj    = tanh(h2 @ W3 + b3) * 0.1
  p      = softmax((base + adj) / clip(temp, .1), -1)       # (B,T,2)
  out    = EMA over T (s_t = .9 s_{t-1} + .1 p_t, s_0 = p_0)

Sharding: data-parallel over batch, 2 rows per core x 8 cores.

Device-side rewrites (all exact up to fp rounding unless noted):
  * softmax over 2 classes -> sigmoid of the logit difference.
  * EMA over each 128-step chunk is a lower-triangular [128,128] matmul
    plus carry matmuls from the previous chunks' last elements.
  * gelu via erf:  2*gelu(x) = x*(1+erf(x/sqrt(2))).  The factor 2 on
    h1g cancels inside LN2 when LN2's eps is scaled 4x; the factor 2 on
    h2g is folded into W3 (host-side W3/2).  This keeps the scalar
    engine inside ONE activation-table set (sigmoid_and_others: copy /
    erf / sigmoid / tanh) -- act-table swaps cost ~1.3us each.
  * rstd = 1/sqrt(var+eps) via fast-inverse-sqrt (magic constant + one
    Newton step, ~0.17% max err -- far inside the output tolerance) on
    the vector engine, batched per group.  The Newton step is signed so
    rstd comes out negative; the sign cancels in the odd-erf identity.
  * mm1 runs in fp8e4 with DoubleRow (2 k-tiles per matmul).  W1 is
    scaled x1024 on the host so its tiny xavier-initialised values sit
    in fp8's normal range; the scale cancels inside LN1 (eps x 2^20).
    The fp8 quantisation of X/W1 perturbs only the +-0.006 tanh
    adjustments -> ~5e-4 relative error on the routing weights, far
    inside the 2e-2 gate.  mm2/mm3 run in bf16; EMA matmuls in fp32.

X is transposed and cast to fp8 on the host into the matmul-ready
lhsT layout [pair, ad%128, chunk%2, ad//128, token]; each 2-chunk pair
is then ONE contiguous HWDGE DMA (512 KB) straight into SBUF -- no
on-device transposes of X and a quarter of the fp32 HBM traffic.
"""

import os
import numpy as np
import ml_dtypes

B, T, AD = 16, 1024, 2048
HID1, HID2 = 256, 128
NCORES = 8
B_LOC = B // NCORES            # 2 rows per core
CH_ROW = T // 128              # 8 chunks per row
CH = B_LOC * CH_ROW            # 16 chunks per core
NPAIR = CH // 2                # 8 two-chunk DMA pairs
GRP = 2                        # chunks per LN/head batch group
KC = AD // 128                 # 16 contraction chunks for mm1
SM = 0.9
ADJ = 0.1
LN_EPS = 1e-5
MAGIC = 0x5f3759df - 0x00400000   # seed for rsqrt of v2 = v/2
W1S = 1024.0                      # host-side W1 scale for fp8

_BF16 = ml_dtypes.bfloat16
_FP8 = ml_dtypes.float8_e4m3fn

_NC = {}
LAST_RESULTS = None


def _make_ema_mats():
    """EMA-as-matmul constants, all pre-transposed to lhsT layout [k, tau].

    s_c = A_loc @ p_c + 0.9^(tau+1) * s_{c-1}[127] and the carry expands
    into rank-1 matmuls against p_{c-1}, p_{c-2}: contributions beyond
    depth 2 carry a 0.9^256 ~ 1.8e-12 factor -> exactly zero in fp32.
    This removes the serial cross-chunk dependency entirely.
    """
    tau = np.arange(128, dtype=np.float64)
    diff = tau[:, None] - tau[None, :]
    Am = np.where(diff >= 0, 0.1 * SM ** diff, 0.0)
    A0 = Am.copy()
    A0[:, 0] = SM ** tau
    dec = SM ** (tau + 1.0)          # 0.9^(tau+1)
    r1f = np.outer(A0[127, :], dec)  # [k, tau], carry from chunk 0
    r1m = np.outer(Am[127, :], dec)
    r2f = (SM ** 128) * r1f
    r2m = (SM ** 128) * r1m
    f32c = lambda a: np.ascontiguousarray(a, np.float32)
    return {
        "a0t": f32c(A0.T), "amt": f32c(Am.T),
        "r1f": f32c(r1f), "r1m": f32c(r1m),
        "r2f": f32c(r2f), "r2m": f32c(r2m),
    }


def _build_nc(sim_gelu=False, triv1=True, triv2=True, trivb3=True,
              inv_t=1.0):
    # trivN: layer-N has b==0, g==1, be==0 (true for this problem's
    # setup_inputs); skips the bias matmul and the affine stt ops.
    # trivb3: b3 == 0.  inv_t: 1/clip(temperature, .1), baked in.
    # sim_gelu: CoreSim has no Erf LUT; substitute Tanh so the identical
    # program structure can run under the simulator (race/OOB checks).
    import concourse.mybir as mybir
    import concourse.tile as tile
    from concourse import bacc

    f32 = mybir.dt.float32
    bf16 = mybir.dt.bfloat16
    fp8 = mybir.dt.float8e4
    i32 = mybir.dt.int32
    AF = mybir.ActivationFunctionType
    OP = mybir.AluOpType
    DR = mybir.MatmulPerfMode.DoubleRow
    ERF = AF.Tanh if sim_gelu else AF.Erf
    INV_SQRT2 = float(1.0 / np.sqrt(2.0))

    nc = bacc.Bacc()

    # ---- DRAM parameters (per-core) ----
    x_d = nc.declare_dram_parameter("x", [NPAIR, 128, 2, KC, 128], fp8,
                                    isOutput=False)
    lab_d = nc.declare_dram_parameter("labels", [CH, 128], i32, isOutput=False)
    w1_d = nc.declare_dram_parameter("w1", [128, KC, HID1], fp8, isOutput=False)
    w2_d = nc.declare_dram_parameter("w2", [128, 2, HID2], bf16, isOutput=False)
    w3_d = nc.declare_dram_parameter("w3", [128, 2], bf16, isOutput=False)
    b1_d = nc.declare_dram_parameter("b1", [1, HID1], bf16, isOutput=False)
    b2_d = nc.declare_dram_parameter("b2", [1, HID2], bf16, isOutput=False)
    b3_d = nc.declare_dram_parameter("b3g", [128, 2 * GRP], f32, isOutput=False)
    g1_d = nc.declare_dram_parameter("g1bn", [128, HID1], f32, isOutput=False)
    be1_d = nc.declare_dram_parameter("be1b", [128, HID1], f32, isOutput=False)
    g2_d = nc.declare_dram_parameter("g2bn", [128, HID2], f32, isOutput=False)
    be2_d = nc.declare_dram_parameter("be2b", [128, HID2], f32, isOutput=False)
    ema_d = {
        name: nc.declare_dram_parameter(name, [128, 128], f32, isOutput=False)
        for name in ("a0t", "amt", "r1f", "r1m", "r2f", "r2m")
    }
    idb_d = nc.declare_dram_parameter("idbf", [128, 128], bf16, isOutput=False)
    idf_d = nc.declare_dram_parameter("idf32", [16, 16], f32, isOutput=False)
    ones_d = nc.declare_dram_parameter("ones1", [1, 128], bf16, isOutput=False)
    out_d = nc.declare_dram_parameter("out", [128, CH, 2], f32, isOutput=True)

    with tile.TileContext(nc) as tc:
        with (
            tc.tile_pool(name="singles", bufs=1) as singles,
            tc.tile_pool(name="xtpool", bufs=4) as xtpool,
            tc.tile_pool(name="act", bufs=4) as act,
            tc.tile_pool(name="hbuf", bufs=8) as hbuf,
            tc.tile_pool(name="stat", bufs=4) as stat,
            tc.tile_pool(name="pp", bufs=1, space="PSUM") as pp,
        ):
            # PSUM budget (8 banks): mm 3 + tp1 2 + tp2 1 + yh 2.
            def load(name, shape, dt, src):
                t = singles.tile(shape, dt, tag=name)
                nc.sync.dma_start(t[:], src[:])
                return t

            xpD = {}

            def s1_dma_pair(p, split=False):
                """one contiguous HWDGE load of a 2-chunk lhsT pair."""
                xt = xtpool.tile([128, 2, KC, 128], fp8, tag="xt")
                if split:   # first pair: land chunk 0 before chunk 1
                    nc.sync.dma_start(xt[:, 0], x_d[p, :, 0])
                    nc.sync.dma_start(xt[:, 1], x_d[p, :, 1])
                else:
                    nc.sync.dma_start(xt[:], x_d[p])
                xpD[p] = xt

            # w1 split in two halves so mm1 of chunk 0 can start after
            # ~half the weight load; chunk-0/1 x is queued between them.
            w1_s = singles.tile([128, KC, HID1], fp8, tag="w1")
            nc.sync.dma_start(w1_s[:, :KC // 2, :], w1_d[:, :KC // 2, :])
            s1_dma_pair(0, split=True)
            nc.sync.dma_start(w1_s[:, KC // 2:, :], w1_d[:, KC // 2:, :])
            idb_s = load("idb", [128, 128], bf16, idb_d)
            ones_s = (None if (triv1 and triv2)
                      else load("ones", [1, 128], bf16, ones_d))
            b1_s = None if triv1 else load("b1", [1, HID1], bf16, b1_d)
            s1_dma_pair(1)
            s1_dma_pair(2)

            idf_s = lh_s = None

            def label_prep():
                nonlocal idf_s, lh_s
                idf_s = load("idf", [16, 16], f32, idf_d)
                lab_i = singles.tile([CH, 128], i32)
                nc.sync.dma_start(lab_i[:], lab_d[:])
                lab_f = singles.tile([CH, 128], f32)
                nc.vector.tensor_copy(lab_f[:], lab_i[:])
                p_lab = pp.tile([128, CH], f32, tag="yh", bufs=2,
                                name="plab")
                nc.tensor.transpose(p_lab[:, :CH], lab_f[:], idf_s[:])
                lh_s = singles.tile([128, CH], f32)
                nc.vector.tensor_scalar(
                    out=lh_s[:], in0=p_lab[:, :CH], scalar1=0.5,
                    scalar2=None, op0=OP.subtract)

            def load_rest():
                nonlocal w2_s, w3_s, b2_s, b3g_s, g1_s, be1_s, g2_s, \
                    be2_s, ema_s
                w2_s = load("w2", [128, 2, HID2], bf16, w2_d)
                w3_s = load("w3", [128, 2], bf16, w3_d)
                b2_s = None if triv2 else load("b2", [1, HID2], bf16, b2_d)
                b3g_s = (None if trivb3
                         else load("b3g", [128, 2 * GRP], f32, b3_d))
                g1_s = be1_s = g2_s = be2_s = None
                if not triv1:
                    g1_s = load("g1", [128, HID1], f32, g1_d)  # holds -g1
                    be1_s = load("be1", [128, HID1], f32, be1_d)
                if not triv2:
                    g2_s = load("g2", [128, HID2], f32, g2_d)  # holds -g2
                    be2_s = load("be2", [128, HID2], f32, be2_d)
                ema_s = {name: load(name, [128, 128], f32, d)
                         for name, d in ema_d.items()}

            w2_s = w3_s = b2_s = b3g_s = g1_s = be1_s = g2_s = be2_s = None
            ema_s = None

            s_all = singles.tile([128, CH, 2], f32)
            pc_full = singles.tile([128, CH, 2], f32)
            magic_s = singles.tile([128, 1], i32)
            nc.vector.memset(magic_s[:], MAGIC)

            def rsqrt_full(var_ap, n, eps, tagsuf):
                """negative 1/sqrt(var+eps) batched over n columns (fast
                inverse sqrt + one Newton step; the sign cancels in the
                odd-erf gelu identity / the negated gains -g1/-g2)."""
                v2 = stat.tile([128, n], f32, tag="v2" + tagsuf)
                nc.vector.tensor_scalar(
                    out=v2[:], in0=var_ap, scalar1=0.5, scalar2=0.5 * eps,
                    op0=OP.mult, op1=OP.add)
                ib = stat.tile([128, n], i32, tag="ib" + tagsuf)
                nc.vector.tensor_scalar(
                    out=ib[:], in0=v2[:].bitcast(i32), scalar1=1,
                    scalar2=None, op0=OP.logical_shift_right)
                y = stat.tile([128, n], f32, tag="y" + tagsuf)
                nc.vector.tensor_tensor(
                    out=y[:].bitcast(i32),
                    in0=magic_s[:].to_broadcast((128, n)), in1=ib[:],
                    op=OP.subtract)          # y0 = +seed
                p = stat.tile([128, n], f32, tag="p" + tagsuf)
                nc.vector.tensor_tensor(out=p[:], in0=y[:], in1=y[:],
                                        op=OP.mult)
                nc.vector.tensor_tensor(out=p[:], in0=p[:], in1=v2[:],
                                        op=OP.mult)
                # y1n = (p - 1.5) * y0 = -y1  (negative rstd)
                nc.vector.scalar_tensor_tensor(
                    out=y[:], in0=p[:], scalar=1.5, in1=y[:],
                    op0=OP.subtract, op1=OP.mult)
                return y

            mv1G, h1sD, rstd1G, sb1G = {}, {}, {}, {}
            mv2G, h2sD, rstd2G, sb2G, pygG = {}, {}, {}, {}, {}

            def s1_chunk(c):
                """mm1 (fp8 DoubleRow) + LN1 stats for one chunk."""
                g, j = divmod(c, GRP)
                if j == 0:
                    mv1G[g] = stat.tile([128, GRP, 2], f32, tag="mv1",
                                        name=f"mv1_{g}")
                mv1 = mv1G[g]
                xt = xpD[c // 2][:, c % 2]
                if c % 2 == 1:
                    del xpD[c // 2]

                ph1 = pp.tile([128, HID1], f32, tag="mm", bufs=3,
                              name=f"ph1_{c}")
                for kk in range(KC // 2):
                    nc.tensor.matmul(
                        ph1[:], xt[:, 2 * kk:2 * kk + 2, :],
                        w1_s[:, 2 * kk:2 * kk + 2, :],
                        start=(kk == 0), stop=(triv1 and kk == KC // 2 - 1),
                        perf_mode=DR)
                if not triv1:
                    nc.tensor.matmul(
                        ph1[:], ones_s[:], b1_s[:], start=False, stop=True)

                h1s = hbuf.tile([128, HID1], bf16, tag="h1s")
                nc.scalar.activation(out=h1s[:], in_=ph1[:], func=AF.Copy)
                st6 = stat.tile([128, 6], f32, tag="st6")
                nc.vector.bn_stats(st6[:], h1s[:])
                nc.vector.bn_aggr(mv1[:, j, :], st6[:])
                h1sD[c] = h1s

            def s2a_chunk(c):
                """LN1 apply -> mm2 -> LN2 stats for one chunk."""
                g, j = divmod(c, GRP)
                if j == 0:
                    # W1 is x1024 on host -> var x 2^20 -> eps x 2^20;
                    # the scale cancels exactly inside LN1.
                    rstd1G[g] = rsqrt_full(mv1G[g][:, :, 1], GRP,
                                           W1S * W1S * LN_EPS, "a")
                    mv2G[g] = stat.tile([128, GRP, 2], f32, tag="mv2",
                                        name=f"mv2_{g}")
                    if triv1:
                        # erf-affine: ef = erf(h1*sc + bc) runs off h1s
                        # directly, in parallel with xn (not after it)
                        sc = stat.tile([128, GRP], f32, tag="sca")
                        nc.vector.tensor_scalar(
                            out=sc[:], in0=rstd1G[g][:], scalar1=INV_SQRT2,
                            scalar2=None, op0=OP.mult)
                        bc = stat.tile([128, GRP], f32, tag="bca")
                        nc.vector.scalar_tensor_tensor(
                            out=bc[:], in0=mv1G[g][:, :, 0],
                            scalar=-INV_SQRT2, in1=rstd1G[g][:],
                            op0=OP.mult, op1=OP.mult)
                        sb1G[g] = (sc, bc)
                mv1, rstd1 = mv1G[g], rstd1G[g]
                h1s = h1sD.pop(c)

                xn = act.tile([128, HID1], bf16, tag="xn")
                if triv1:
                    sc, bc = sb1G[g]
                    ef = act.tile([128, HID1], bf16, tag="ef")
                    nc.scalar.activation(out=ef[:], in_=h1s[:], func=ERF,
                                         scale=sc[:, j:j + 1],
                                         bias=bc[:, j:j + 1])
                    # xn = (h1 - mu) * (-rstd) = -LN(h1): one 2x-mode
                    # tensor_scalar; the sign cancels in the odd-erf
                    # gelu identity below.
                    nc.vector.tensor_scalar(
                        out=xn[:], in0=h1s[:], scalar1=mv1[:, j, 0:1],
                        scalar2=rstd1[:, j:j + 1],
                        op0=OP.subtract, op1=OP.mult)
                    sgn = -1.0
                else:
                    nc.vector.scalar_tensor_tensor(
                        out=xn[:], in0=h1s[:], scalar=mv1[:, j, 0:1],
                        in1=g1_s[:], op0=OP.subtract, op1=OP.mult)
                    nc.vector.scalar_tensor_tensor(
                        out=xn[:], in0=xn[:], scalar=rstd1[:, j:j + 1],
                        in1=be1_s[:], op0=OP.mult, op1=OP.add)
                    ef = act.tile([128, HID1], bf16, tag="ef")
                    nc.scalar.activation(out=ef[:], in_=xn[:], func=ERF,
                                         scale=INV_SQRT2)
                    sgn = 1.0
                h1g = act.tile([128, HID1], bf16, tag="h1g")
                # 2*gelu(z) = (erf(z/sqrt2) + sgn) * xn  with xn=sgn*z
                nc.vector.scalar_tensor_tensor(
                    out=h1g[:], in0=ef[:], scalar=sgn, in1=xn[:],
                    op0=OP.add, op1=OP.mult)

                pt1 = pp.tile([128, 256], bf16, tag="tp1", bufs=2,
                              name=f"pt1_{c}")
                for k in range(2):
                    nc.tensor.transpose(
                        pt1[:, 128 * k:128 * (k + 1)],
                        h1g[:, 128 * k:128 * (k + 1)],
                        idb_s[:])
                h1t = act.tile([128, 2, 128], bf16, tag="h1t")
                nc.scalar.activation(
                    out=h1t[:], in_=pt1[:], func=AF.Copy)

                ph2 = pp.tile([128, HID1], f32, tag="mm", bufs=3,
                              name=f"ph2_{c}")
                for k in range(2):
                    nc.tensor.matmul(
                        ph2[:, :HID2], h1t[:, k, :], w2_s[:, k, :],
                        start=(k == 0), stop=(triv2 and k == 1))
                if not triv2:
                    nc.tensor.matmul(
                        ph2[:, :HID2], ones_s[:], b2_s[:], start=False,
                        stop=True)

                h2s = hbuf.tile([128, HID2], bf16, tag="h2s")
                nc.scalar.activation(out=h2s[:], in_=ph2[:, :HID2],
                                     func=AF.Copy)
                st6b = stat.tile([128, 6], f32, tag="st6")
                nc.vector.bn_stats(st6b[:], h2s[:])
                nc.vector.bn_aggr(mv2G[g][:, j, :], st6b[:])
                h2sD[c] = h2s

            def s2b_chunk(c):
                """LN2 apply -> mm3 -> y for one chunk."""
                g, j = divmod(c, GRP)
                if j == 0:
                    # LN2 eps is 4x because h1g carries the factor 2
                    rstd2G[g] = rsqrt_full(mv2G[g][:, :, 1], GRP,
                                           4.0 * LN_EPS, "b")
                    pygG[g] = pp.tile([128, 4 * GRP], f32, tag="yh",
                                      bufs=2, name=f"yg_{g}")
                    if triv2:
                        sc2 = stat.tile([128, GRP], f32, tag="scb")
                        nc.vector.tensor_scalar(
                            out=sc2[:], in0=rstd2G[g][:],
                            scalar1=INV_SQRT2, scalar2=None, op0=OP.mult)
                        bc2 = stat.tile([128, GRP], f32, tag="bcb")
                        nc.vector.scalar_tensor_tensor(
                            out=bc2[:], in0=mv2G[g][:, :, 0],
                            scalar=-INV_SQRT2, in1=rstd2G[g][:],
                            op0=OP.mult, op1=OP.mult)
                        sb2G[g] = (sc2, bc2)
                mv2, rstd2, pyg = mv2G[g], rstd2G[g], pygG[g]
                h2s = h2sD.pop(c)

                xn2 = act.tile([128, HID2], bf16, tag="xn2")
                if triv2:
                    sc2, bc2 = sb2G[g]
                    ef2 = act.tile([128, HID2], bf16, tag="ef2")
                    nc.scalar.activation(out=ef2[:], in_=h2s[:], func=ERF,
                                         scale=sc2[:, j:j + 1],
                                         bias=bc2[:, j:j + 1])
                    nc.vector.tensor_scalar(
                        out=xn2[:], in0=h2s[:], scalar1=mv2[:, j, 0:1],
                        scalar2=rstd2[:, j:j + 1],
                        op0=OP.subtract, op1=OP.mult)
                    sgn2 = -1.0
                else:
                    nc.vector.scalar_tensor_tensor(
                        out=xn2[:], in0=h2s[:], scalar=mv2[:, j, 0:1],
                        in1=g2_s[:], op0=OP.subtract, op1=OP.mult)
                    nc.vector.scalar_tensor_tensor(
                        out=xn2[:], in0=xn2[:], scalar=rstd2[:, j:j + 1],
                        in1=be2_s[:], op0=OP.mult, op1=OP.add)
                    ef2 = act.tile([128, HID2], bf16, tag="ef2")
                    nc.scalar.activation(out=ef2[:], in_=xn2[:], func=ERF,
                                         scale=INV_SQRT2)
                    sgn2 = 1.0
                h2g = act.tile([128, HID2], bf16, tag="h2g")
                nc.vector.scalar_tensor_tensor(
                    out=h2g[:], in0=ef2[:], scalar=sgn2, in1=xn2[:],
                    op0=OP.add, op1=OP.mult)

                pt2 = pp.tile([128, 128], bf16, tag="tp2", bufs=1,
                              name=f"pt2_{c}")
                nc.tensor.transpose(pt2[:], h2g[:], idb_s[:])
                h2t = act.tile([128, 128], bf16, tag="h2t")
                nc.scalar.activation(out=h2t[:], in_=pt2[:], func=AF.Copy)
                # mm3 writes straight into this group's y columns in PSUM
                nc.tensor.matmul(pyg[:, 2 * j:2 * j + 2], h2t[:], w3_s[:],
                                 start=True, stop=True)

            def head_ema(g):
                """batched head + EMA matmuls for one group."""
                pyg = pygG.pop(g)
                if not trivb3:
                    nc.vector.tensor_tensor(
                        out=pyg[:, :2 * GRP], in0=pyg[:, :2 * GRP],
                        in1=b3g_s[:], op=OP.add)
                th = stat.tile([128, GRP, 2], f32, tag="th")
                nc.scalar.activation(
                    out=th[:].rearrange("p g n -> p (g n)"),
                    in_=pyg[:, :2 * GRP], func=AF.Tanh)
                dcol = stat.tile([128, GRP], f32, tag="dcol")
                nc.vector.tensor_tensor(
                    out=dcol[:], in0=th[:, :, 1], in1=th[:, :, 0],
                    op=OP.subtract)
                nc.vector.scalar_tensor_tensor(
                    out=dcol[:], in0=dcol[:], scalar=ADJ,
                    in1=lh_s[:, GRP * g:GRP * (g + 1)],
                    op0=OP.mult, op1=OP.add)
                pc = pc_full[:, GRP * g:GRP * (g + 1), :]
                nc.scalar.activation(
                    out=pc[:, :, 1], in_=dcol[:], func=AF.Sigmoid,
                    scale=float(inv_t))
                # p0 = 1 - p1 (exact identity for sigmoid)
                nc.vector.tensor_scalar(
                    out=pc[:, :, 0], in0=pc[:, :, 1], scalar1=-1.0,
                    scalar2=1.0, op0=OP.mult, op1=OP.add)

                # EMA: group-batched matmuls, no serial dep; the EMA
                # outputs land in cols 4:8 of the same yh bank.
                cs = GRP * g
                if (cs % CH_ROW) == 0:
                    # row start: chunk cs uses A0; cs+1 carries from it
                    mms = [("a0t", cs, 1, 0, True),
                           ("amt", cs + 1, 1, 2, True),
                           ("r1f", cs, 1, 2, False)]
                elif (cs % CH_ROW) == 2:
                    # cs-2 is the row's A0 chunk -> r2f for cs's carry-2
                    mms = [("amt", cs, 2, 0, True),
                           ("r1m", cs - 1, 2, 0, False),
                           ("r2f", cs - 2, 1, 0, False),
                           ("r2m", cs - 1, 1, 2, False)]
                else:
                    mms = [("amt", cs, 2, 0, True),
                           ("r1m", cs - 1, 2, 0, False),
                           ("r2m", cs - 2, 2, 0, False)]
                for i, (mat, c0, n, off, st) in enumerate(mms):
                    nc.tensor.matmul(
                        pyg[:, 4 + off:4 + off + 2 * n], ema_s[mat][:],
                        pc_full[:, c0:c0 + n, :],
                        start=st, stop=(i == len(mms) - 1),
                        skip_group_check=True)
                nc.vector.tensor_copy(
                    out=s_all[:, cs:cs + GRP, :],
                    in_=pyg[:, 4:8].rearrange("p (c n) -> p c n", n=2))
                if (cs + GRP) % CH_ROW == 0:   # row done -> one 64B/line DMA
                    r = cs // CH_ROW
                    nc.sync.dma_start(
                        out=out_d[:, CH_ROW * r:CH_ROW * (r + 1), :],
                        in_=s_all[:, CH_ROW * r:CH_ROW * (r + 1), :])

            # chunk-granular software pipeline: stage offsets keep every
            # engine's in-order stream dense instead of draining group by
            # group at the end.  X pairs are prefetched ~5 chunks ahead.
            D2A, D2B, DHE = 2, 4, 6
            NG = CH // GRP
            s1_chunk(0)
            for t in range(1, CH + DHE + 1):
                if t % 2 == 1 and t + 5 < CH:
                    s1_dma_pair((t + 5) // 2)
                if t == 1:
                    load_rest()
                if t == 2:
                    label_prep()
                if t < CH:
                    s1_chunk(t)
                if 0 <= t - D2A < CH:
                    s2a_chunk(t - D2A)
                if 0 <= t - D2B < CH:
                    s2b_chunk(t - D2B)
                if t >= DHE and (t - DHE) % GRP == 0 and (t - DHE) // GRP < NG:
                    head_ema((t - DHE) // GRP)

    if not sim_gelu:
        nc.compile()   # bacc pass pipeline (regalloc, wait splitting, ...)
    return nc


def _get_nc(triv1=True, triv2=True, trivb3=True, inv_t=1.0):
    key = (triv1, triv2, trivb3, float(inv_t))
    if key not in _NC:
        _NC[key] = _build_nc(triv1=triv1, triv2=triv2, trivb3=trivb3,
                             inv_t=inv_t)
    return _NC[key]


def _host_inputs(inputs):
    """Build the per-core input maps from the full problem inputs."""
    x = np.asarray(inputs["action_tokens"], np.float32)
    labels = np.asarray(inputs["critical_labels"]).astype(np.int32)
    W1 = np.asarray(inputs["W1"], np.float32)
    W2 = np.asarray(inputs["W2"], np.float32)
    W3 = np.asarray(inputs["W3"], np.float32)
    b1 = np.asarray(inputs["b1"], np.float32)
    b2 = np.asarray(inputs["b2"], np.float32)
    b3 = np.asarray(inputs["b3"], np.float32)
    g1 = np.asarray(inputs["g1"], np.float32)
    be1 = np.asarray(inputs["be1"], np.float32)
    g2 = np.asarray(inputs["g2"], np.float32)
    be2 = np.asarray(inputs["be2"], np.float32)

    ema = _make_ema_mats()

    # X -> fp8 lhsT pair layout [b, pair, ad%128, chunk%2, ad//128, tok];
    # each per-core pair is then one contiguous 512 KB HWDGE DMA.
    xt = np.ascontiguousarray(
        x.reshape(B, NPAIR // B_LOC, 2, 128, KC, 128)
         .transpose(0, 1, 5, 2, 4, 3)
    ).astype(_FP8)

    w1p = np.ascontiguousarray(
        (W1S * W1).reshape(KC, 128, HID1).transpose(1, 0, 2)).astype(_FP8)
    w2p = np.ascontiguousarray(
        W2.reshape(2, 128, HID2).transpose(1, 0, 2)).astype(_BF16)
    # h2g carries a factor 2 (erf-gelu without the 0.5) -> fold into W3
    w3p = (0.5 * W3).astype(_BF16)
    # h1g carries a factor 2 -> h2 = h1g'@W2 + 2*b2, LN2 eps scaled 4x
    b2p = (2.0 * b2).reshape(1, HID2).astype(_BF16)

    shared = {
        "w1": w1p,
        "w2": w2p,
        "w3": w3p,
        "b1": (W1S * b1).reshape(1, HID1).astype(_BF16),
        "b2": b2p,
        "b3g": np.broadcast_to(np.tile(b3, GRP), (128, 2 * GRP))
                .astype(np.float32).copy(),
        # negated gains: the device-side rstd is negative (see rsqrt_full)
        "g1bn": np.broadcast_to(-g1, (128, HID1)).copy(),
        "be1b": np.broadcast_to(be1, (128, HID1)).copy(),
        "g2bn": np.broadcast_to(-g2, (128, HID2)).copy(),
        "be2b": np.broadcast_to(be2, (128, HID2)).copy(),
        **ema,
        "idbf": np.eye(128, dtype=_BF16),
        "idf32": np.eye(16, dtype=np.float32),
        "ones1": np.ones((1, 128), dtype=_BF16),
    }

    in_maps = []
    for core in range(NCORES):
        r0 = core * B_LOC
        m = dict(shared)
        m["x"] = np.ascontiguousarray(
            xt[r0:r0 + B_LOC].reshape(NPAIR, 128, 2, KC, 128))
        m["labels"] = np.ascontiguousarray(
            labels[r0:r0 + B_LOC].reshape(CH, 128))
        in_maps.append(m)
    return in_maps


def kernel(**inputs) -> np.ndarray:
    global LAST_RESULTS
    from concourse.bass_utils import run_bass_kernel_spmd

    triv1 = (not np.any(np.asarray(inputs["b1"]))
             and np.all(np.asarray(inputs["g1"]) == 1)
             and not np.any(np.asarray(inputs["be1"])))
    triv2 = (not np.any(np.asarray(inputs["b2"]))
             and np.all(np.asarray(inputs["g2"]) == 1)
             and not np.any(np.asarray(inputs["be2"])))
    trivb3 = not np.any(np.asarray(inputs["b3"]))
    temp = float(np.asarray(inputs["temperature"]))
    inv_t = 1.0 / max(temp, 0.1)
    nc = _get_nc(triv1, triv2, trivb3, inv_t)
    in_maps = _host_inputs(inputs)
    trace = bool(int(os.environ.get("BLSR_TRACE", "0")))
    res = run_bass_kernel_spmd(
        nc, in_maps, list(range(NCORES)), trace=trace)
    LAST_RESULTS = res
    # device output is [128, CH, 2] = [tau, row*8+chunk, class]
    out = np.empty((B, T, 2), np.float32)
    for core in range(NCORES):
        st = res.results[core]["out"].reshape(128, B_LOC, CH_ROW, 2)
        out[core * B_LOC:(core + 1) * B_LOC] = (
            st.transpose(1, 2, 0, 3).reshape(B_LOC, T, 2))
    return out.astype(np.float32)


# revision 22
# speedup vs baseline: 1.3006x; 1.3006x over previous
"""Trainium2 Bass kernel for nn_BinaryLabelSoftRouter.

Reference computation (B=16, T=1024, D=2048, H=256, H2=128):
  base   = where(labels>0, [.25,.75], [.75,.25])            # (B,T,2)
  h1     = gelu(LN(x @ W1 + b1) * g1 + be1)                 # erf gelu
  h2     = gelu(LN(h1 @ W2 + b2) * g2 + be2)
  adj    = tanh(h2 @ W3 + b3) * 0.1
  p      = softmax((base + adj) / clip(temp, .1), -1)       # (B,T,2)
  out    = EMA over T (s_t = .9 s_{t-1} + .1 p_t, s_0 = p_0)

Sharding: data-parallel over batch, 2 rows per core x 8 cores.

Device-side rewrites (all exact up to fp rounding unless noted):
  * softmax over 2 classes -> sigmoid of the logit difference.
  * EMA over each 128-step chunk is a lower-triangular [128,128] matmul
    plus carry matmuls from the previous chunks' last elements.
  * gelu via erf:  2*gelu(x) = x*(1+erf(x/sqrt(2))).  The factor 2 on
    h1g cancels inside LN2 when LN2's eps is scaled 4x; the factor 2 on
    h2g is folded into W3 (host-side W3/2).  This keeps the scalar
    engine inside ONE activation-table set (sigmoid_and_others: copy /
    erf / sigmoid / tanh) -- act-table swaps cost ~1.3us each.
  * rstd = 1/sqrt(var+eps) via fast-inverse-sqrt (magic constant + one
    Newton step, ~0.17% max err -- far inside the output tolerance) on
    the vector engine, batched per group.  The Newton step is signed so
    rstd comes out negative; the sign cancels in the odd-erf identity.
  * mm1 runs in fp8e4 with DoubleRow (2 k-tiles per matmul).  W1 is
    scaled x1024 on the host so its tiny xavier-initialised values sit
    in fp8's normal range; the scale cancels inside LN1 (eps x 2^20).
    The fp8 quantisation of X/W1 perturbs only the +-0.006 tanh
    adjustments -> ~5e-4 relative error on the routing weights, far
    inside the 2e-2 gate.  mm2/mm3 run in bf16; EMA matmuls in fp32.

X is transposed and cast to fp8 on the host into the matmul-ready
lhsT layout [pair, ad%128, chunk%2, ad//128, token]; each 2-chunk pair
is then ONE contiguous HWDGE DMA (512 KB) straight into SBUF -- no
on-device transposes of X and a quarter of the fp32 HBM traffic.
"""

import os
import numpy as np
import ml_dtypes

B, T, AD = 16, 1024, 2048
HID1, HID2 = 256, 128
NCORES = 8
B_LOC = B // NCORES            # 2 rows per core
CH_ROW = T // 128              # 8 chunks per row
CH = B_LOC * CH_ROW            # 16 chunks per core
NPAIR = CH // 2                # 8 two-chunk DMA pairs
GRP = 4                        # chunks per LN/head batch group
KC = AD // 128                 # 16 contraction chunks for mm1
SM = 0.9
ADJ = 0.1
LN_EPS = 1e-5
MAGIC = 0x5f3759df - 0x00400000   # seed for rsqrt of v2 = v/2

_BF16 = ml_dtypes.bfloat16
_FP8 = ml_dtypes.float8_e4m3fn

_NC = {}
LAST_RESULTS = None


def _make_ema_mats():
    """EMA-as-matmul constants, all pre-transposed to lhsT layout [k, tau].

    s_c = A_loc @ p_c + 0.9^(tau+1) * s_{c-1}[127] and the carry expands
    into rank-1 matmuls against p_{c-1}, p_{c-2}: contributions beyond
    depth 2 carry a 0.9^256 ~ 1.8e-12 factor -> exactly zero in fp32.
    This removes the serial cross-chunk dependency entirely.
    """
    tau = np.arange(128, dtype=np.float64)
    diff = tau[:, None] - tau[None, :]
    Am = np.where(diff >= 0, 0.1 * SM ** diff, 0.0)
    A0 = Am.copy()
    A0[:, 0] = SM ** tau
    dec = SM ** (tau + 1.0)          # 0.9^(tau+1)
    r1f = np.outer(A0[127, :], dec)  # [k, tau], carry from chunk 0
    r1m = np.outer(Am[127, :], dec)
    r2f = (SM ** 128) * r1f
    r2m = (SM ** 128) * r1m
    f32c = lambda a: np.ascontiguousarray(a, np.float32)
    return {
        "a0t": f32c(A0.T), "amt": f32c(Am.T),
        "r1f": f32c(r1f), "r1m": f32c(r1m),
        "r2f": f32c(r2f), "r2m": f32c(r2m),
    }


def _build_nc(sim_gelu=False, triv1=True, triv2=True, trivb3=True,
              inv_t=1.0):
    # trivN: layer-N has b==0, g==1, be==0 (true for this problem's
    # setup_inputs); skips the bias matmul and the affine stt ops.
    # trivb3: b3 == 0.  inv_t: 1/clip(temperature, .1), baked in.
    # sim_gelu: CoreSim has no Erf LUT; substitute Tanh so the identical
    # program structure can run under the simulator (race/OOB checks).
    import concourse.mybir as mybir
    import concourse.tile as tile
    from concourse import bacc

    f32 = mybir.dt.float32
    bf16 = mybir.dt.bfloat16
    fp8 = mybir.dt.float8e4
    i32 = mybir.dt.int32
    AF = mybir.ActivationFunctionType
    OP = mybir.AluOpType
    DR = mybir.MatmulPerfMode.DoubleRow
    ERF = AF.Tanh if sim_gelu else AF.Erf
    INV_SQRT2 = float(1.0 / np.sqrt(2.0))

    nc = bacc.Bacc()

    # ---- DRAM parameters (per-core) ----
    x_d = nc.declare_dram_parameter("x", [NPAIR, 128, 2, KC, 128], fp8,
                                    isOutput=False)
    lab_d = nc.declare_dram_parameter("labels", [CH, 128], i32, isOutput=False)
    w1_d = nc.declare_dram_parameter("w1", [128, KC, HID1], bf16, isOutput=False)
    w2_d = nc.declare_dram_parameter("w2", [128, 2, HID2], bf16, isOutput=False)
    w3_d = nc.declare_dram_parameter("w3", [128, 2], bf16, isOutput=False)
    b1_d = nc.declare_dram_parameter("b1", [1, HID1], bf16, isOutput=False)
    b2_d = nc.declare_dram_parameter("b2", [1, HID2], bf16, isOutput=False)
    b3_d = nc.declare_dram_parameter("b3g", [128, 2 * GRP], f32, isOutput=False)
    g1_d = nc.declare_dram_parameter("g1bn", [128, HID1], f32, isOutput=False)
    be1_d = nc.declare_dram_parameter("be1b", [128, HID1], f32, isOutput=False)
    g2_d = nc.declare_dram_parameter("g2bn", [128, HID2], f32, isOutput=False)
    be2_d = nc.declare_dram_parameter("be2b", [128, HID2], f32, isOutput=False)
    ema_d = {
        name: nc.declare_dram_parameter(name, [128, 128], f32, isOutput=False)
        for name in ("a0t", "amt", "r1f", "r1m", "r2f", "r2m")
    }
    idb_d = nc.declare_dram_parameter("idbf", [128, 128], bf16, isOutput=False)
    idf_d = nc.declare_dram_parameter("idf32", [16, 16], f32, isOutput=False)
    ones_d = nc.declare_dram_parameter("ones1", [1, 128], bf16, isOutput=False)
    out_d = nc.declare_dram_parameter("out", [128, CH, 2], f32, isOutput=True)

    with tile.TileContext(nc) as tc:
        with (
            tc.tile_pool(name="singles", bufs=1) as singles,
            tc.tile_pool(name="xtpool", bufs=4) as xtpool,
            tc.tile_pool(name="act", bufs=4) as act,
            tc.tile_pool(name="hbuf", bufs=8) as hbuf,
            tc.tile_pool(name="stat", bufs=4) as stat,
            tc.tile_pool(name="pp", bufs=1, space="PSUM") as pp,
        ):
            # PSUM budget (8 banks): mm 3 + tp1 2 + tp2 1 + yh 2.
            def load(name, shape, dt, src):
                t = singles.tile(shape, dt, tag=name)
                nc.sync.dma_start(t[:], src[:])
                return t

            xpD = {}

            def s1_dma_pair(p, split=False):
                """one contiguous HWDGE load of a 2-chunk lhsT pair."""
                xt = xtpool.tile([128, 2, KC, 128], fp8, tag="xt")
                if split:   # first pair: land chunk 0 before chunk 1
                    nc.sync.dma_start(xt[:, 0], x_d[p, :, 0])
                    nc.sync.dma_start(xt[:, 1], x_d[p, :, 1])
                else:
                    nc.sync.dma_start(xt[:], x_d[p])
                xpD[p] = xt

            # idb first (warm-up matmuls need it asap), then w1 in two
            # halves so mm1 of chunk 0 can start after ~half the weight
            # load; chunk-0/1 x is queued between them.
            idb_s = load("idb", [128, 128], bf16, idb_d)
            w1_s = singles.tile([128, KC, HID1], bf16, tag="w1")
            nc.sync.dma_start(w1_s[:, :KC // 2, :], w1_d[:, :KC // 2, :])
            s1_dma_pair(0, split=True)
            nc.sync.dma_start(w1_s[:, KC // 2:, :], w1_d[:, KC // 2:, :])
            ones_s = (None if (triv1 and triv2)
                      else load("ones", [1, 128], bf16, ones_d))
            b1_s = None if triv1 else load("b1", [1, HID1], bf16, b1_d)
            s1_dma_pair(1)
            s1_dma_pair(2)

            idf_s = lh_s = None

            def label_prep():
                nonlocal idf_s, lh_s
                idf_s = load("idf", [16, 16], f32, idf_d)
                lab_i = singles.tile([CH, 128], i32)
                nc.sync.dma_start(lab_i[:], lab_d[:])
                lab_f = singles.tile([CH, 128], f32)
                nc.vector.tensor_copy(lab_f[:], lab_i[:])
                p_lab = pp.tile([128, CH], f32, tag="yh", bufs=2,
                                name="plab")
                nc.tensor.transpose(p_lab[:, :CH], lab_f[:], idf_s[:])
                lh_s = singles.tile([128, CH], f32)
                nc.vector.tensor_scalar(
                    out=lh_s[:], in0=p_lab[:, :CH], scalar1=0.5,
                    scalar2=None, op0=OP.subtract)

            def load_rest():
                nonlocal w2_s, w3_s, b2_s, b3g_s, g1_s, be1_s, g2_s, \
                    be2_s, ema_s
                w2_s = load("w2", [128, 2, HID2], bf16, w2_d)
                w3_s = load("w3", [128, 2], bf16, w3_d)
                b2_s = None if triv2 else load("b2", [1, HID2], bf16, b2_d)
                b3g_s = (None if trivb3
                         else load("b3g", [128, 2 * GRP], f32, b3_d))
                g1_s = be1_s = g2_s = be2_s = None
                if not triv1:
                    g1_s = load("g1", [128, HID1], f32, g1_d)  # holds -g1
                    be1_s = load("be1", [128, HID1], f32, be1_d)
                if not triv2:
                    g2_s = load("g2", [128, HID2], f32, g2_d)  # holds -g2
                    be2_s = load("be2", [128, HID2], f32, be2_d)
                ema_s = {name: load(name, [128, 128], f32, d)
                         for name, d in ema_d.items()}

            w2_s = w3_s = b2_s = b3g_s = g1_s = be1_s = g2_s = be2_s = None
            ema_s = None

            s_all = singles.tile([128, CH, 2], f32)
            pc_full = singles.tile([128, CH, 2], f32)
            magic_s = singles.tile([128, 1], i32)
            nc.vector.memset(magic_s[:], MAGIC)

            def rsqrt_full(var_ap, n, eps, tagsuf):
                """negative 1/sqrt(var+eps) batched over n columns (fast
                inverse sqrt + one Newton step; the sign cancels in the
                odd-erf gelu identity / the negated gains -g1/-g2)."""
                v2 = stat.tile([128, n], f32, tag="v2" + tagsuf)
                nc.vector.tensor_scalar(
                    out=v2[:], in0=var_ap, scalar1=0.5, scalar2=0.5 * eps,
                    op0=OP.mult, op1=OP.add)
                ib = stat.tile([128, n], i32, tag="ib" + tagsuf)
                nc.vector.tensor_scalar(
                    out=ib[:], in0=v2[:].bitcast(i32), scalar1=1,
                    scalar2=None, op0=OP.logical_shift_right)
                y = stat.tile([128, n], f32, tag="y" + tagsuf)
                nc.vector.tensor_tensor(
                    out=y[:].bitcast(i32),
                    in0=magic_s[:].to_broadcast((128, n)), in1=ib[:],
                    op=OP.subtract)          # y0 = +seed
                p = stat.tile([128, n], f32, tag="p" + tagsuf)
                nc.vector.tensor_tensor(out=p[:], in0=y[:], in1=y[:],
                                        op=OP.mult)
                nc.vector.tensor_tensor(out=p[:], in0=p[:], in1=v2[:],
                                        op=OP.mult)
                # y1n = (p - 1.5) * y0 = -y1  (negative rstd)
                nc.vector.scalar_tensor_tensor(
                    out=y[:], in0=p[:], scalar=1.5, in1=y[:],
                    op0=OP.subtract, op1=OP.mult)
                return y

            mv1G, h1sD, rstd1G, sb1G = {}, {}, {}, {}
            mv2G, h2sD, rstd2G, sb2G, pygG = {}, {}, {}, {}, {}

            def s1_chunk(c):
                """mm1 (fp8 DoubleRow) + LN1 stats for one chunk."""
                g, j = divmod(c, GRP)
                if j == 0:
                    mv1G[g] = stat.tile([128, GRP, 2], f32, tag="mv1",
                                        name=f"mv1_{g}")
                mv1 = mv1G[g]
                xt = xpD[c // 2][:, c % 2]
                if c % 2 == 1:
                    del xpD[c // 2]

                ph1 = pp.tile([128, HID1], f32, tag="mm", bufs=3,
                              name=f"ph1_{c}")
                for k in range(KC):
                    nc.tensor.matmul(
                        ph1[:], xt[:, k, :], w1_s[:, k, :],
                        start=(k == 0), stop=(triv1 and k == KC - 1))
                if not triv1:
                    nc.tensor.matmul(
                        ph1[:], ones_s[:], b1_s[:], start=False, stop=True)

                h1s = hbuf.tile([128, HID1], bf16, tag="h1s")
                nc.scalar.activation(out=h1s[:], in_=ph1[:], func=AF.Copy)
                st6 = stat.tile([128, 6], f32, tag="st6")
                nc.vector.bn_stats(st6[:], h1s[:])
                nc.vector.bn_aggr(mv1[:, j, :], st6[:])
                h1sD[c] = h1s

            def s2a_chunk(c):
                """LN1 apply -> mm2 -> LN2 stats for one chunk."""
                g, j = divmod(c, GRP)
                if j == 0:
                    rstd1G[g] = rsqrt_full(mv1G[g][:, :, 1], GRP,
                                           LN_EPS, "a")
                    mv2G[g] = stat.tile([128, GRP, 2], f32, tag="mv2",
                                        name=f"mv2_{g}")
                mv1, rstd1 = mv1G[g], rstd1G[g]
                h1s = h1sD.pop(c)

                xn = act.tile([128, HID1], bf16, tag="xn")
                if triv1:
                    # xn = (h1 - mu) * (-rstd) = -LN(h1): one 2x-mode
                    # tensor_scalar; the sign cancels in the odd-erf
                    # gelu identity below.
                    nc.vector.tensor_scalar(
                        out=xn[:], in0=h1s[:], scalar1=mv1[:, j, 0:1],
                        scalar2=rstd1[:, j:j + 1],
                        op0=OP.subtract, op1=OP.mult)
                    sgn = -1.0
                else:
                    nc.vector.scalar_tensor_tensor(
                        out=xn[:], in0=h1s[:], scalar=mv1[:, j, 0:1],
                        in1=g1_s[:], op0=OP.subtract, op1=OP.mult)
                    nc.vector.scalar_tensor_tensor(
                        out=xn[:], in0=xn[:], scalar=rstd1[:, j:j + 1],
                        in1=be1_s[:], op0=OP.mult, op1=OP.add)
                    sgn = 1.0
                ef = act.tile([128, HID1], bf16, tag="ef")
                nc.scalar.activation(out=ef[:], in_=xn[:], func=ERF,
                                     scale=INV_SQRT2)
                h1g = act.tile([128, HID1], bf16, tag="h1g")
                # 2*gelu(z) = (erf(z/sqrt2) + sgn) * xn  with xn=sgn*z
                nc.vector.scalar_tensor_tensor(
                    out=h1g[:], in0=ef[:], scalar=sgn, in1=xn[:],
                    op0=OP.add, op1=OP.mult)

                pt1 = pp.tile([128, 256], bf16, tag="tp1", bufs=2,
                              name=f"pt1_{c}")
                for k in range(2):
                    nc.tensor.transpose(
                        pt1[:, 128 * k:128 * (k + 1)],
                        h1g[:, 128 * k:128 * (k + 1)],
                        idb_s[:])
                h1t = act.tile([128, 2, 128], bf16, tag="h1t")
                nc.scalar.activation(
                    out=h1t[:], in_=pt1[:], func=AF.Copy)

                ph2 = pp.tile([128, HID1], f32, tag="mm", bufs=3,
                              name=f"ph2_{c}")
                for k in range(2):
                    nc.tensor.matmul(
                        ph2[:, :HID2], h1t[:, k, :], w2_s[:, k, :],
                        start=(k == 0), stop=(triv2 and k == 1))
                if not triv2:
                    nc.tensor.matmul(
                        ph2[:, :HID2], ones_s[:], b2_s[:], start=False,
                        stop=True)

                h2s = hbuf.tile([128, HID2], bf16, tag="h2s")
                nc.scalar.activation(out=h2s[:], in_=ph2[:, :HID2],
                                     func=AF.Copy)
                st6b = stat.tile([128, 6], f32, tag="st6")
                nc.vector.bn_stats(st6b[:], h2s[:])
                nc.vector.bn_aggr(mv2G[g][:, j, :], st6b[:])
                h2sD[c] = h2s

            def s2b_chunk(c):
                """LN2 apply -> mm3 -> y for one chunk."""
                g, j = divmod(c, GRP)
                if j == 0:
                    # LN2 eps is 4x because h1g carries the factor 2
                    rstd2G[g] = rsqrt_full(mv2G[g][:, :, 1], GRP,
                                           4.0 * LN_EPS, "b")
                    pygG[g] = pp.tile([128, 4 * GRP], f32, tag="yh",
                                      bufs=2, name=f"yg_{g}")
                mv2, rstd2, pyg = mv2G[g], rstd2G[g], pygG[g]
                h2s = h2sD.pop(c)

                xn2 = act.tile([128, HID2], bf16, tag="xn2")
                if triv2:
                    nc.vector.tensor_scalar(
                        out=xn2[:], in0=h2s[:], scalar1=mv2[:, j, 0:1],
                        scalar2=rstd2[:, j:j + 1],
                        op0=OP.subtract, op1=OP.mult)
                    sgn2 = -1.0
                else:
                    nc.vector.scalar_tensor_tensor(
                        out=xn2[:], in0=h2s[:], scalar=mv2[:, j, 0:1],
                        in1=g2_s[:], op0=OP.subtract, op1=OP.mult)
                    nc.vector.scalar_tensor_tensor(
                        out=xn2[:], in0=xn2[:], scalar=rstd2[:, j:j + 1],
                        in1=be2_s[:], op0=OP.mult, op1=OP.add)
                    sgn2 = 1.0
                ef2 = act.tile([128, HID2], bf16, tag="ef2")
                nc.scalar.activation(out=ef2[:], in_=xn2[:], func=ERF,
                                     scale=INV_SQRT2)
                h2g = act.tile([128, HID2], bf16, tag="h2g")
                nc.vector.scalar_tensor_tensor(
                    out=h2g[:], in0=ef2[:], scalar=sgn2, in1=xn2[:],
                    op0=OP.add, op1=OP.mult)

                pt2 = pp.tile([128, 128], bf16, tag="tp2", bufs=1,
                              name=f"pt2_{c}")
                nc.tensor.transpose(pt2[:], h2g[:], idb_s[:])
                h2t = act.tile([128, 128], bf16, tag="h2t")
                nc.scalar.activation(out=h2t[:], in_=pt2[:], func=AF.Copy)
                # mm3 writes straight into this group's y columns in PSUM
                nc.tensor.matmul(pyg[:, 2 * j:2 * j + 2], h2t[:], w3_s[:],
                                 start=True, stop=True)

            def head_ema(g):
                """batched head + EMA matmuls for one group."""
                pyg = pygG.pop(g)
                if not trivb3:
                    nc.vector.tensor_tensor(
                        out=pyg[:, :2 * GRP], in0=pyg[:, :2 * GRP],
                        in1=b3g_s[:], op=OP.add)
                th = stat.tile([128, GRP, 2], f32, tag="th")
                nc.scalar.activation(
                    out=th[:].rearrange("p g n -> p (g n)"),
                    in_=pyg[:, :2 * GRP], func=AF.Tanh)
                dcol = stat.tile([128, GRP], f32, tag="dcol")
                nc.vector.tensor_tensor(
                    out=dcol[:], in0=th[:, :, 1], in1=th[:, :, 0],
                    op=OP.subtract)
                nc.vector.scalar_tensor_tensor(
                    out=dcol[:], in0=dcol[:], scalar=ADJ,
                    in1=lh_s[:, GRP * g:GRP * (g + 1)],
                    op0=OP.mult, op1=OP.add)
                pc = pc_full[:, GRP * g:GRP * (g + 1), :]
                nc.scalar.activation(
                    out=pc[:, :, 1], in_=dcol[:], func=AF.Sigmoid,
                    scale=float(inv_t))
                # p0 = 1 - p1 (exact identity for sigmoid)
                nc.vector.tensor_scalar(
                    out=pc[:, :, 0], in0=pc[:, :, 1], scalar1=-1.0,
                    scalar2=1.0, op0=OP.mult, op1=OP.add)

                # EMA: group-batched matmuls, no serial dep; the EMA
                # outputs land in cols 8:16 of the same yh bank.
                cs = GRP * g
                if (cs % CH_ROW) == 0:
                    # chunks cc=0..3 of a row: chunk 0 uses A0 / feeds R*f
                    mms = [("a0t", cs, 1, 0, True),
                           ("amt", cs + 1, 3, 2, True),
                           ("r1f", cs, 1, 2, False),
                           ("r1m", cs + 1, 2, 4, False),
                           ("r2f", cs, 1, 4, False),
                           ("r2m", cs + 1, 1, 6, False)]
                else:
                    mms = [("amt", cs, 4, 0, True),
                           ("r1m", cs - 1, 4, 0, False),
                           ("r2m", cs - 2, 4, 0, False)]
                for i, (mat, c0, n, off, st) in enumerate(mms):
                    nc.tensor.matmul(
                        pyg[:, 8 + off:8 + off + 2 * n], ema_s[mat][:],
                        pc_full[:, c0:c0 + n, :],
                        start=st, stop=(i == len(mms) - 1),
                        skip_group_check=True)
                nc.vector.tensor_copy(
                    out=s_all[:, cs:cs + GRP, :],
                    in_=pyg[:, 8:16].rearrange("p (c n) -> p c n", n=2))
                if (cs + GRP) % CH_ROW == 0:   # row done -> one 64B/line DMA
                    r = cs // CH_ROW
                    nc.sync.dma_start(
                        out=out_d[:, CH_ROW * r:CH_ROW * (r + 1), :],
                        in_=s_all[:, CH_ROW * r:CH_ROW * (r + 1), :])

            # HAM warm-up: real matmuls (transposes don't count as
            # PE-busy for the activity monitor) on the already-resident
            # identity while the first x/w1 DMAs land, so mm1 starts at
            # 2.4 GHz instead of ramping from the 1.2 GHz cold clock.
            wjunk = pp.tile([128, 128], f32, tag="tp2", bufs=1,
                            name="warm")
            for _ in range(32):
                nc.tensor.matmul(wjunk[:], idb_s[:], idb_s[:],
                                 start=True, stop=True)

            # chunk-granular software pipeline: stage offsets keep every
            # engine's in-order stream dense instead of draining group by
            # group at the end.  X pairs are prefetched ~5 chunks ahead.
            D2A, D2B, DHE = 4, 8, 12
            NG = CH // GRP
            s1_chunk(0)
            for t in range(1, CH + DHE + 1):
                if t % 2 == 1 and t + 5 < CH:
                    s1_dma_pair((t + 5) // 2)
                if t == 1:
                    load_rest()
                if t == 2:
                    label_prep()
                if t < CH:
                    s1_chunk(t)
                if 0 <= t - D2A < CH:
                    s2a_chunk(t - D2A)
                if 0 <= t - D2B < CH:
                    s2b_chunk(t - D2B)
                if t >= DHE and (t - DHE) % GRP == 0 and (t - DHE) // GRP < NG:
                    head_ema((t - DHE) // GRP)

    if not sim_gelu:
        nc.compile()   # bacc pass pipeline (regalloc, wait splitting, ...)
    return nc


def _get_nc(triv1=True, triv2=True, trivb3=True, inv_t=1.0):
    key = (triv1, triv2, trivb3, float(inv_t))
    if key not in _NC:
        _NC[key] = _build_nc(triv1=triv1, triv2=triv2, trivb3=trivb3,
                             inv_t=inv_t)
    return _NC[key]


def _host_inputs(inputs):
    """Build the per-core input maps from the full problem inputs."""
    x = np.asarray(inputs["action_tokens"], np.float32)
    labels = np.asarray(inputs["critical_labels"]).astype(np.int32)
    W1 = np.asarray(inputs["W1"], np.float32)
    W2 = np.asarray(inputs["W2"], np.float32)
    W3 = np.asarray(inputs["W3"], np.float32)
    b1 = np.asarray(inputs["b1"], np.float32)
    b2 = np.asarray(inputs["b2"], np.float32)
    b3 = np.asarray(inputs["b3"], np.float32)
    g1 = np.asarray(inputs["g1"], np.float32)
    be1 = np.asarray(inputs["be1"], np.float32)
    g2 = np.asarray(inputs["g2"], np.float32)
    be2 = np.asarray(inputs["be2"], np.float32)

    ema = _make_ema_mats()

    # X -> fp8 lhsT pair layout [b, pair, ad%128, chunk%2, ad//128, tok];
    # each per-core pair is then one contiguous 512 KB HWDGE DMA.
    xt = np.ascontiguousarray(
        x.reshape(B, NPAIR // B_LOC, 2, 128, KC, 128)
         .transpose(0, 1, 5, 2, 4, 3)
    ).astype(_FP8)

    w1p = np.ascontiguousarray(
        W1.reshape(KC, 128, HID1).transpose(1, 0, 2)).astype(_BF16)
    w2p = np.ascontiguousarray(
        W2.reshape(2, 128, HID2).transpose(1, 0, 2)).astype(_BF16)
    # h2g carries a factor 2 (erf-gelu without the 0.5) -> fold into W3
    w3p = (0.5 * W3).astype(_BF16)
    # h1g carries a factor 2 -> h2 = h1g'@W2 + 2*b2, LN2 eps scaled 4x
    b2p = (2.0 * b2).reshape(1, HID2).astype(_BF16)

    shared = {
        "w1": w1p,
        "w2": w2p,
        "w3": w3p,
        "b1": b1.reshape(1, HID1).astype(_BF16),
        "b2": b2p,
        "b3g": np.broadcast_to(np.tile(b3, GRP), (128, 2 * GRP))
                .astype(np.float32).copy(),
        # negated gains: the device-side rstd is negative (see rsqrt_full)
        "g1bn": np.broadcast_to(-g1, (128, HID1)).copy(),
        "be1b": np.broadcast_to(be1, (128, HID1)).copy(),
        "g2bn": np.broadcast_to(-g2, (128, HID2)).copy(),
        "be2b": np.broadcast_to(be2, (128, HID2)).copy(),
        **ema,
        "idbf": np.eye(128, dtype=_BF16),
        "idf32": np.eye(16, dtype=np.float32),
        "ones1": np.ones((1, 128), dtype=_BF16),
    }

    in_maps = []
    for core in range(NCORES):
        r0 = core * B_LOC
        m = dict(shared)
        m["x"] = np.ascontiguousarray(
            xt[r0:r0 + B_LOC].reshape(NPAIR, 128, 2, KC, 128))
        m["labels"] = np.ascontiguousarray(
            labels[r0:r0 + B_LOC].reshape(CH, 128))
        in_maps.append(m)
    return in_maps


def kernel(**inputs) -> np.ndarray:
    global LAST_RESULTS
    from concourse.bass_utils import run_bass_kernel_spmd

    triv1 = (not np.any(np.asarray(inputs["b1"]))
             and np.all(np.asarray(inputs["g1"]) == 1)
             and not np.any(np.asarray(inputs["be1"])))
    triv2 = (not np.any(np.asarray(inputs["b2"]))
             and np.all(np.asarray(inputs["g2"]) == 1)
             and not np.any(np.asarray(inputs["be2"])))
    trivb3 = not np.any(np.asarray(inputs["b3"]))
    temp = float(np.asarray(inputs["temperature"]))
    inv_t = 1.0 / max(temp, 0.1)
    nc = _get_nc(triv1, triv2, trivb3, inv_t)
    in_maps = _host_inputs(inputs)
    trace = bool(int(os.environ.get("BLSR_TRACE", "0")))
    res = run_bass_kernel_spmd(
        nc, in_maps, list(range(NCORES)), trace=trace)
    LAST_RESULTS = res
    # device output is [128, CH, 2] = [tau, row*8+chunk, class]
    out = np.empty((B, T, 2), np.float32)
    for core in range(NCORES):
        st = res.results[core]["out"].reshape(128, B_LOC, CH_ROW, 2)
        out[core * B_LOC:(core + 1) * B_LOC] = (
            st.transpose(1, 2, 0, 3).reshape(B_LOC, T, 2))
    return out.astype(np.float32)
